# revision 39
# baseline (speedup 1.0000x reference)
"""Mesa-layer memory kernel for Trainium2 (8 NeuronCores, data-parallel over B).

Math: the reference's T-step Sherman-Morrison / discounted-accumulation
recurrence has a closed form,
    R_final = (I + K^T K)^{-1}            (eps term is O(1e-6) relative)
    S_final^T = K^T diag(c) V,   c_t = prod_{s>t} gamma_s
so per memory b the output is
    out_b = Q_b @ (R_b @ S_b^T).
R is inverted with 5 Newton-Schulz iterations in residual form
    X <- X + X^T (I - A X)
run entirely in fp16 (1 cycle/row on the PE, 10 mantissa bits; validated
1.2e-3 max-rel vs fp64 closed form in numpy simulation, 16x under the
2e-2 gate). The output is stored to HBM in fp16 as well, halving the
output traffic; the host upcasts.

Layout trick: timestep t maps to (partition p, slot r) via t = 16 p + r,
making every DMA a fully contiguous 8 KB-per-partition transfer.

The suffix cumprod of gammas runs in log space: 16-step free-dim scans
plus one triangular matmul for the cross-partition prefix.

Emission is software-pipelined so the DMA stream never stalls: the A/S
contractions of memories 4-7 are interleaved into the serial dependency
gaps of group 0's Newton-Schulz iterations, and group 0's readout is
interleaved with group 1's iterations. K/V loads are issued ahead of Q
loads on the same queue so the recurrence-critical data arrives first;
Q0/Q1 are hoisted into the K/V stream so group 0's transposes can start
during the load phase. Copy work is spread over Scalar/DVE/GpSimd.

Each core owns B/8 = 8 independent memories; no cross-core communication.
"""

import numpy as np

B, T, DK, DV, NQ = 64, 2048, 128, 128, 2048
NCORES = 8
BPC = B // NCORES          # memories per core
P = 128                    # partitions
R16 = T // P               # 16 row-slots per partition
GCLAMP = 1e-30             # gamma clamp before log (exact-0 gammas)
NS_IT = 5                  # Newton-Schulz iterations (all fp16)
NGRP = 2
GSZ = BPC // NGRP


def build_nc(ns_it=NS_IT):
    import concourse.mybir as mybir
    import concourse.tile as tile
    from concourse import bacc
    from concourse.masks import make_identity, make_upper_triangular

    fp32 = mybir.dt.float32
    fp16 = mybir.dt.float16
    AF = mybir.ActivationFunctionType
    OP = mybir.AluOpType
    AX = mybir.AxisListType

    nc = bacc.Bacc(trn_type="TRN2", target_bir_lowering=False, debug=False)
    keys = nc.dram_tensor("keys", [BPC, T, DK], fp32, kind="ExternalInput").ap()
    values = nc.dram_tensor("values", [BPC, T, DV], fp32, kind="ExternalInput").ap()
    gammas = nc.dram_tensor("gammas", [BPC, T], fp32, kind="ExternalInput").ap()
    queries = nc.dram_tensor("queries", [BPC, NQ, DK], fp32, kind="ExternalInput").ap()
    out = nc.dram_tensor("out", [BPC, NQ, DV], fp16, kind="ExternalOutput").ap()

    with tile.TileContext(nc) as tc:
        const = tc.alloc_tile_pool(name="const", bufs=1)
        gam = tc.alloc_tile_pool(name="gam", bufs=1)
        kp = tc.alloc_tile_pool(name="kp", bufs=3)
        vp = tc.alloc_tile_pool(name="vp", bufs=3)
        kvbp = tc.alloc_tile_pool(name="kvbp", bufs=2)
        qp = tc.alloc_tile_pool(name="qp", bufs=BPC)
        qhp = tc.alloc_tile_pool(name="qhp", bufs=4)
        qtp = tc.alloc_tile_pool(name="qtp", bufs=3)
        small = tc.alloc_tile_pool(name="small", bufs=1)
        xs = tc.alloc_tile_pool(name="xs", bufs=2)
        outp = tc.alloc_tile_pool(name="outp", bufs=3)
        ps_sm = tc.alloc_tile_pool(name="ps_sm", bufs=2, space="PSUM")
        ps_qt = tc.alloc_tile_pool(name="ps_qt", bufs=2, space="PSUM")
        ps_rd = tc.alloc_tile_pool(name="ps_rd", bufs=2, space="PSUM")
        ps_ns = tc.alloc_tile_pool(name="ps_ns", bufs=2, space="PSUM")

        ident = const.tile([P, P], fp32)
        make_identity(nc, ident)
        ident_h = const.tile([P, P], fp16)
        make_identity(nc, ident_h)
        ident4 = const.tile([P, GSZ * P], fp32)
        for i in range(GSZ):
            make_identity(nc, ident4[:, i * P : (i + 1) * P])
        utri = const.tile([P, P], fp32)
        make_upper_triangular(nc, utri, val=1.0, diag=False)
        ones2 = const.tile([P, P], fp32)
        nc.gpsimd.memset(ones2[:], 1.0)

        # ---- DMA issue: gammas first (tiny), then K/V with Q0/Q1 hoisted
        # into the stream, then the remaining Qs ----
        g16 = gam.tile([P, BPC, R16], fp32)
        nc.sync.dma_start(g16[:], gammas.rearrange("i (p r) -> p i r", r=R16))
        k_sb = [None] * BPC
        v_sb = [None] * BPC
        q_sb = [None] * BPC

        def qdma(i):
            q_sb[i] = qp.tile([P, R16, DK], fp32, tag="q", name=f"q{i}")
            nc.sync.dma_start(q_sb[i][:], queries[i].rearrange("(p r) k -> p r k", p=P))

        for i in range(BPC):
            k_sb[i] = kp.tile([P, R16, DK], fp32, tag="k", name=f"k{i}")
            nc.sync.dma_start(k_sb[i][:], keys[i].rearrange("(p r) k -> p r k", p=P))
            v_sb[i] = vp.tile([P, R16, DV], fp32, tag="v", name=f"v{i}")
            nc.sync.dma_start(v_sb[i][:], values[i].rearrange("(p r) k -> p r k", p=P))
            if i == 3:
                qdma(0)
            if i == 5:
                qdma(1)
        for i in range(2, BPC):
            qdma(i)

        # ---- phase 0: suffix cumprod of gammas (log space) ----
        g16f = g16.rearrange("p i r -> p (i r)")
        nc.vector.tensor_scalar_max(g16f, g16f, GCLAMP)
        nc.scalar.activation(g16f, g16f, AF.Ln)
        incl = gam.tile([P, BPC, R16], fp32)
        zz = gam.tile([P, R16], fp32)
        nc.vector.memset(zz[:], 0.0)
        # joiner: make DVE observe the ACT (Ln) dependency before the scans
        joiner = gam.tile([P, 1], fp32)
        nc.vector.tensor_copy(out=joiner[:], in_=g16[:, 0, 0:1])
        for i in range(BPC):
            nc.vector.tensor_tensor_scan(
                incl[:, i, :], g16[:, i, :], zz[:], 0.0, OP.add, OP.add
            )
        ptot = gam.tile([P, BPC], fp32)
        nc.vector.tensor_copy(out=ptot[:], in_=incl[:, :, R16 - 1])
        ps_pre = ps_sm.tile([P, 2 * BPC], fp32, tag="sm", name="ps_pre")
        nc.tensor.matmul(ps_pre[:, 0:BPC], utri[:], ptot[:])
        nc.tensor.matmul(ps_pre[:, BPC : 2 * BPC], ones2[:], ptot[:])
        pre_sb = gam.tile([P, 2 * BPC], fp32)
        nc.vector.tensor_copy(out=pre_sb[:], in_=ps_pre[:])
        bias2 = gam.tile([P, BPC], fp32)
        nc.vector.tensor_tensor(
            bias2[:], pre_sb[:, BPC : 2 * BPC], pre_sb[:, 0:BPC], OP.subtract
        )
        c_t = gam.tile([P, BPC, R16], fp32)
        for i in range(BPC):
            nc.scalar.activation(
                c_t[:, i, :], incl[:, i, :], AF.Exp,
                bias=bias2[:, i : i + 1], scale=-1.0,
            )

        # ---- per-memory state tiles ----
        A_lp = [small.tile([P, P], fp16, tag=f"A{i}", name=f"A{i}") for i in range(BPC)]
        ST_lp = [small.tile([P, P], fp16, tag=f"S{i}", name=f"S{i}") for i in range(BPC)]
        Phi_lp = [small.tile([P, P], fp16, tag=f"P{i}", name=f"Phi{i}") for i in range(BPC)]
        rs_sb = [small.tile([P, 1], fp32, tag=f"r{i}", name=f"rs{i}") for i in range(BPC)]
        qt_sb = [None] * BPC
        Xg = [None] * NGRP

        def prep(i):
            """kv build (fp16) + A/S contraction + A_lp/ST/rs for memory i."""
            kvb = kvbp.tile([P, R16, 2 * P], fp16, tag="kvb", name=f"kvb{i}")
            nc.scalar.copy(out=kvb[:, :, 0:DK], in_=k_sb[i][:])
            # alternate the V*c multiply between DVE and GpSimd
            veng = nc.vector if i % 2 == 0 else nc.gpsimd
            veng.tensor_tensor(
                kvb[:, :, DK : 2 * DK], v_sb[i][:],
                c_t[:, i, :, None].to_broadcast((P, R16, DV)),
                OP.mult,
            )
            ps = ps_sm.tile([P, 2 * P], fp32, tag="sm", name=f"ps_as{i}")
            for r in range(R16):
                nc.tensor.matmul(
                    ps[:], kvb[:, r, 0:DK], kvb[:, r, :],
                    start=(r == 0), stop=(r == R16 - 1),
                )
            nc.vector.tensor_tensor(A_lp[i][:], ps[:, 0:P], ident[:], OP.add)
            nc.scalar.copy(out=ST_lp[i][:], in_=ps[:, P : 2 * P])
            nc.vector.tensor_reduce(
                rs_sb[i][:], A_lp[i][:], AX.X, OP.add, apply_absolute_value=True
            )
            nc.vector.reciprocal(rs_sb[i][:], rs_sb[i][:])

        def x0(g):
            xw = xs.tile([P, GSZ * P], fp16, tag=f"X{g}", name=f"X{g}_0")
            for i in range(GSZ):
                nc.gpsimd.tensor_tensor(
                    xw[:, i * P : (i + 1) * P], ident[:],
                    rs_sb[GSZ * g + i][:].to_broadcast((P, P)),
                    OP.mult,
                )
            Xg[g] = xw

        eg_sb = [None] * NGRP

        def ns_a(g, it):
            """pa = A @ X (4 matmuls) + eg = I - pa (DVE)."""
            pa = ps_ns.tile([P, GSZ * P], fp32, tag="ns", name=f"pa{g}_{it}")
            for i in range(GSZ):
                sl = slice(i * P, (i + 1) * P)
                nc.tensor.matmul(pa[:, sl], A_lp[GSZ * g + i][:], Xg[g][:, sl])
            eg = xs.tile([P, GSZ * P], fp16, tag=f"e{g}", name=f"e{g}_{it}")
            nc.vector.scalar_tensor_tensor(
                eg[:], pa[:], -1.0, ident4[:], OP.mult, OP.add
            )
            eg_sb[g] = eg

        def ns_b(g, it):
            """pb = X @ eg (4 matmuls) + X' = X + pb (DVE)."""
            pb = ps_ns.tile([P, GSZ * P], fp32, tag="ns", name=f"pb{g}_{it}")
            for i in range(GSZ):
                sl = slice(i * P, (i + 1) * P)
                nc.tensor.matmul(pb[:, sl], Xg[g][:, sl], eg_sb[g][:, sl])
            xn = xs.tile([P, GSZ * P], fp16, tag=f"X{g}", name=f"X{g}_{it + 1}")
            nc.vector.tensor_tensor(xn[:], Xg[g][:], pb[:], OP.add)
            Xg[g] = xn

        def phi(i):
            g, sl = i // GSZ, slice((i % GSZ) * P, (i % GSZ + 1) * P)
            ps_phi = ps_sm.tile([P, P], fp32, tag="sm", name=f"ps_phi{i}")
            nc.tensor.matmul(ps_phi[:], Xg[g][:, sl], ST_lp[i][:])
            nc.scalar.copy(out=Phi_lp[i][:], in_=ps_phi[:])

        qh_sb = [None] * BPC
        o_tiles = [None] * BPC
        ps_late = [None]

        def qcast(i):
            """Cast Q_i to fp16 as soon as it lands (frees the fp32 slot)."""
            qh_sb[i] = qhp.tile([P, R16, DK], fp16, tag="qh", name=f"qh{i}")
            nc.scalar.copy(out=qh_sb[i][:], in_=q_sb[i][:])

        def qt_chunk(i, h):
            """Transpose 8 Q slots of memory i on the PE (one PSUM bank)."""
            if h == 0:
                qt_sb[i] = qtp.tile([P, R16, P], fp16, tag="qt", name=f"qt{i}")
            ps_q = ps_qt.tile([P, 8 * P], fp16, tag="qt", name=f"ps_qt{i}_{h}")
            for j in range(8):
                nc.tensor.transpose(
                    ps_q[:, j * P : (j + 1) * P], qh_sb[i][:, 8 * h + j, :],
                    ident_h[:],
                )
            nc.vector.tensor_copy(
                out=qt_sb[i][:, 8 * h : 8 * h + 8, :], in_=ps_q[:]
            )

        def ro_chunk(i, r4):
            """Apply Phi to 4 transposed Q slots; store each half as it lands."""
            if r4 == 0:
                o_tiles[i] = outp.tile([P, R16, DV], fp16, tag="o", name=f"o{i}")
            o_sb = o_tiles[i]
            pool = ps_late[0] if ps_late[0] is not None else ps_rd
            ps_o = pool.tile([P, 4 * P], fp32, tag="rd", name=f"ps_o{i}_{r4}")
            for j in range(4):
                nc.tensor.matmul(
                    ps_o[:, j * P : (j + 1) * P], qt_sb[i][:, 4 * r4 + j, :],
                    Phi_lp[i][:],
                )
            # split the PSUM->SBUF cast copies between Scalar and DVE
            if r4 % 2 == 0:
                nc.scalar.copy(out=o_sb[:, 4 * r4 : 4 * r4 + 4, :], in_=ps_o[:])
            else:
                nc.vector.tensor_copy(out=o_sb[:, 4 * r4 : 4 * r4 + 4, :], in_=ps_o[:])
            if r4 == R16 // 4 - 1:
                nc.gpsimd.dma_start(
                    out[i].rearrange("(p r) v -> p r v", p=P), o_sb[:]
                )

        # ---- pipelined emission ----
        for i in range(4):
            prep(i)
        x0(0)
        ns_a(0, 0)
        prep(4)
        ns_b(0, 0)
        ns_a(0, 1)
        qcast(0)
        prep(5)
        ns_b(0, 1)
        ns_a(0, 2)
        qt_chunk(0, 0)
        prep(6)
        ns_b(0, 2)
        ns_a(0, 3)
        qt_chunk(0, 1)
        qcast(1)
        prep(7)
        x0(1)
        ns_b(0, 3)
        ns_a(0, 4)
        qt_chunk(1, 0)
        ns_a(1, 0)
        ns_b(0, 4)
        qt_chunk(1, 1)
        ns_b(1, 0)
        for i in range(4):
            phi(i)
        qcast(2)
        ns_a(1, 1)
        ro_chunk(0, 0); ro_chunk(0, 1)
        ns_b(1, 1)
        ro_chunk(0, 2); qt_chunk(2, 0); ro_chunk(0, 3); qt_chunk(2, 1)
        qcast(3)
        ns_a(1, 2)
        ro_chunk(1, 0); ro_chunk(1, 1)
        ns_b(1, 2)
        ro_chunk(1, 2); qt_chunk(3, 0); ro_chunk(1, 3); qt_chunk(3, 1)
        qcast(4)
        ns_a(1, 3)
        ro_chunk(2, 0); ro_chunk(2, 1)
        ns_b(1, 3)
        ro_chunk(2, 2); qt_chunk(4, 0); ro_chunk(2, 3); qt_chunk(4, 1)
        qcast(5)
        ns_a(1, 4)
        ro_chunk(3, 0); ro_chunk(3, 1)
        ns_b(1, 4)
        for i in range(4, 8):
            phi(i)
        # NS PSUM banks are dead; recycle them so the late readout has
        # twice the PSUM depth and the PE stops stalling on slot recycling
        ps_ns.release()
        ps_late[0] = tc.alloc_tile_pool(name="ps_late", bufs=2, space="PSUM")
        ro_chunk(3, 2); qt_chunk(5, 0); ro_chunk(3, 3); qt_chunk(5, 1)
        qcast(6)
        ro_chunk(4, 0); ro_chunk(4, 1)
        ro_chunk(4, 2); qt_chunk(6, 0); ro_chunk(4, 3); qt_chunk(6, 1)
        qcast(7)
        ro_chunk(5, 0); ro_chunk(5, 1)
        ro_chunk(5, 2); qt_chunk(7, 0); ro_chunk(5, 3); qt_chunk(7, 1)
        for i in range(6, 8):
            ro_chunk(i, 0); ro_chunk(i, 1); ro_chunk(i, 2); ro_chunk(i, 3)
        for pool in (ps_late[0], ps_rd, ps_qt, ps_sm, outp, xs, small, qtp,
                     qhp, qp, kvbp, vp, kp, gam, const):
            pool.release()

    if not nc.is_finalized():
        nc.finalize()
    return nc


def kernel(**inputs) -> np.ndarray:
    keys = np.ascontiguousarray(inputs["keys"], dtype=np.float32)
    values = np.ascontiguousarray(inputs["values"], dtype=np.float32)
    gammas = np.ascontiguousarray(inputs["gammas"], dtype=np.float32)
    queries = np.ascontiguousarray(inputs["queries"], dtype=np.float32)

    from concourse.bass_utils import run_bass_kernel_spmd

    nc = build_nc()
    in_maps = []
    for m in range(NCORES):
        s = slice(m * BPC, (m + 1) * BPC)
        in_maps.append(
            {
                "keys": keys[s],
                "values": values[s],
                "gammas": gammas[s],
                "queries": queries[s],
            }
        )
    res = run_bass_kernel_spmd(nc, in_maps, core_ids=list(range(NCORES)))
    return np.concatenate(
        [res.results[m]["out"] for m in range(NCORES)], axis=0
    ).astype(np.float32)
